# revision 24
# baseline (speedup 1.0000x reference)
"""Trainium2 Bass kernel for BehaviorLemming, v4.

Two fused stencil steps, data-parallel over batch (B=16 / 8 cores).
Layout: H rows in partitions, (channel, W) in free dim, 5 groups of 4ch.

v4 vs v3:
- World uploaded as fp16 from the host (plus an fp32 density channel,
  W-padded): the on-device fp32->fp16 conversion pass disappears and
  input DMA traffic roughly halves.
- All products and mask-chain ops emitted as scalar_tensor_tensor /
  tensor_scalar (InstTensorScalarPtr): 4x DVE mode for all-fp16 SBUF
  operands, 2x for fp32 -- twice the throughput of tensor_tensor.
- Density comparisons read W-padded tiles ([np, 514]) so each compare
  is a single full-width op (no wrap-piece ops).
- b16 (mask roll) and dA2 (density roll) via SBUF->SBUF DMA partition
  shifts on the main sets instead of PE matmuls; exact step-1 density
  via pure DVE adds (no fp32 matmuls). PE runs only the fp16 chains.
- Step-1 stay term folded in on DVE (w1b += R) instead of the I@R
  matmul chain: balances PE vs DVE.
"""

import numpy as np

import concourse.bacc as bacc
import concourse.mybir as mybir
import concourse.tile as tile
from concourse.bass_utils import run_bass_kernel_spmd

B, C, H, W = 16, 20, 512, 512
WP = W + 2
N_CORES = 8
B_PER_CORE = B // N_CORES
ELEM_ID = 3.0
F32 = mybir.dt.float32
F16 = mybir.dt.float16
U16 = mybir.dt.uint16
NCH = 4
NGRP = C // NCH
MAIN_OUT = 124
M_B1 = 32              # partition offset of batch-1 block in the merged set
M_NP = 52

N_SETS = 9


def QMODE(idx, step, g):
    """Q product split: channels on DVE vs Pool (mult is the only fast
    legal Pool op). Returns n channels on DVE (0..4)."""
    if idx >= N_SETS - 2:
        return 4          # tail: Pool drains slower
    if g in (0, 2, 4):
        return 0
    return 1


def RMODE(idx, step, g):
    """Stay-term via PE identity-matmul chain ("pe") or DVE post-add on
    the evacuated tile ("dve"). Merged set: PE-bound drain, DVE idle."""
    if idx == N_SETS - 1:
        return "dve"
    if idx == N_SETS - 2 and step == 2:
        return "dve"
    return "pe"

# knob: engine for the exact-density multiplies ("pool" or "dve")
EXACT_MULT_ENG = "pool"

# knob: engine for the density compares ("pool" or "dve")
CMP_ENG = "dve"

# knob: engine for the mask AND-chain and exact-density adds
CHAIN_ENG = "dve"
EXACT_ADD_ENG = "pool"


def _load_rows(nc, dst_tile, src_ap, row_start, n_rows, p0=0, nch=NCH):
    """Load n_rows (mod H, split at wrap) of src [nch,H,W] into dst
    partitions [p0, p0+n_rows), free dim = (c, w)."""
    s = row_start % H
    remaining = n_rows
    while remaining > 0:
        n = min(remaining, H - s)
        src = src_ap[:, s : s + n, :].rearrange("c h w -> h c w")
        nc.sync.dma_start(out=dst_tile[p0 : p0 + n, :].rearrange(
            "h (c w) -> h c w", c=nch), in_=src)
        p0 += n
        s = (s + n) % H
        remaining -= n


def _load_rows_2d(nc, dst_tile, src_ap, row_start, n_rows, p0=0,
                  c0=0, c1=WP):
    """Load n_rows (mod H) of src [H, WP] cols [c0,c1) into dst
    partitions [p0, p0+n_rows)."""
    s = row_start % H
    remaining = n_rows
    while remaining > 0:
        n = min(remaining, H - s)
        nc.sync.dma_start(out=dst_tile[p0 : p0 + n, 0 : c1 - c0],
                          in_=src_ap[s : s + n, c0:c1])
        p0 += n
        s = (s + n) % H
        remaining -= n


class SetCtx:
    """Per-set emission state."""

    def __init__(self, sd):
        self.sd = sd
        self.wb = None        # [np,10240] f16 world
        self.d32 = None       # [np,514] f32 padded density
        self.dA1 = None       # [np,514] f32 padded density, rows -1
        self.dB1 = None       # [np,512] f32 density, rows +1
        self.w1b = None       # [np,10240] f16 step-1 world
        self.w1dp = None      # [np,514] f32 exact padded step-1 density
        self.m1 = None        # (mp, b16) step-1 masks
        self.m2 = None


def build_kernel():
    nc = bacc.Bacc("TRN2", target_bir_lowering=False, debug=False,
                   num_devices=N_CORES)
    wd16 = nc.dram_tensor("world16", [B_PER_CORE, C, H, W], F16,
                          kind="ExternalInput").ap()
    dpad = nc.dram_tensor("dpad32", [B_PER_CORE, H, WP], F32,
                          kind="ExternalInput").ap()
    su16_d = nc.dram_tensor("su16", [128, 128], F16, kind="ExternalInput").ap()
    sd16_d = nc.dram_tensor("sd16", [128, 128], F16, kind="ExternalInput").ap()
    i16_d = nc.dram_tensor("i16", [128, 128], F16, kind="ExternalInput").ap()
    sum16_d = nc.dram_tensor("sum16", [M_NP, M_NP], F16,
                             kind="ExternalInput").ap()
    sdm16_d = nc.dram_tensor("sdm16", [M_NP, M_NP], F16,
                             kind="ExternalInput").ap()
    sum32_d = nc.dram_tensor("sum32", [M_NP, M_NP], F32,
                             kind="ExternalInput").ap()
    wm16_d = nc.dram_tensor("wm16", [M_NP, C, W], F16,
                            kind="ExternalInput").ap()
    dm32_d = nc.dram_tensor("dm32", [M_NP, WP], F32,
                            kind="ExternalInput").ap()
    dAm32_d = nc.dram_tensor("dAm32", [M_NP, WP], F32,
                             kind="ExternalInput").ap()
    dBm32_d = nc.dram_tensor("dBm32", [M_NP, W], F32,
                             kind="ExternalInput").ap()
    od = nc.dram_tensor("out16", [B_PER_CORE, C, H, W], F16,
                        kind="ExternalOutput").ap()

    al = mybir.AluOpType

    with tile.TileContext(nc) as tc:
        with (
            tc.tile_pool(name="const", bufs=1) as cpool,
            tc.tile_pool(name="dfp", bufs=2) as dpool,
            tc.tile_pool(name="wbp", bufs=2) as wbpool,
            tc.tile_pool(name="w1p", bufs=2) as w1pool,
            tc.tile_pool(name="mkp", bufs=2) as mkpool,
            tc.tile_pool(name="pqr", bufs=3) as pqrpool,
            tc.tile_pool(name="ogp", bufs=2) as ogpool,
            tc.tile_pool(name="pmain", bufs=4, space="PSUM") as pmain,
        ):
            su16 = cpool.tile([128, 128], F16)
            sd16 = cpool.tile([128, 128], F16)
            i16 = cpool.tile([128, 128], F16)
            sum16 = cpool.tile([M_NP, M_NP], F16)
            sdm16 = cpool.tile([M_NP, M_NP], F16)
            sum32 = cpool.tile([M_NP, M_NP], F32)
            z16 = cpool.tile([128, W], F16)
            z32 = cpool.tile([128, WP], F32)
            nc.gpsimd.memset(z16[0:1, :], 0.0)
            nc.gpsimd.memset(z32[0:1, :], 0.0)

            def load_consts():
                # Act-queue HWDGE: runs in parallel with the set-0 world
                # loads on SP
                for t, d in ((su16, su16_d), (sd16, sd16_d), (i16, i16_d),
                             (sum16, sum16_d), (sdm16, sdm16_d),
                             (sum32, sum32_d)):
                    nc.scalar.dma_start(out=t[:], in_=d)

            def stt(eng, out, in0, in1, op0=al.mult, op1=al.mult,
                    scalar=1.0):
                eng.scalar_tensor_tensor(out=out, in0=in0, scalar=scalar,
                                         in1=in1, op0=op0, op1=op1)

            def masks_rolls(st, step):
                """Phase 1: d/dA sources. Step 2: dA2 = roll(w1d,+1,H)."""
                sd = st.sd
                np_ = sd["np"]
                if step == 1:
                    return {"d": st.d32, "dA": st.dA1}
                d = st.w1dp
                dAt = mkpool.tile([128, WP], F32, tag="dA2")
                if sd.get("merged"):
                    # two matmuls: a matmul output may not cross the
                    # 512-fp32 PSUM bank boundary
                    psx = pmain.tile([np_, 2 * W], F32, tag="ps")
                    nc.tensor.matmul(out=psx[:, 0:W], lhsT=sum32[:],
                                     rhs=d[0:np_, 0:W], start=True,
                                     stop=True)
                    nc.tensor.matmul(out=psx[:, W : W + 2], lhsT=sum32[:],
                                     rhs=d[0:np_, W:WP], start=True,
                                     stop=True)
                    nc.scalar.copy(dAt[0:np_, :], psx[:, 0:WP])
                else:
                    # partition shift via SBUF->SBUF DMA on the DVE queue
                    # (w1dp is DVE-produced: no head-of-line stall); row 0
                    # zeroed -- its masks are never stored but must stay
                    # finite (NaN would poison matmul accumulation).
                    nc.sync.dma_start(out=dAt[1:128, :], in_=d[0:127, :])
                    nc.sync.dma_start(out=dAt[0:1, :], in_=z32[0:1, :])
                return {"d": d, "dA": dAt}

            def masks_cmps(st, step, mc):
                """Phase 2: density compares on W-padded tiles."""
                np_ = st.sd["np"]
                lo = 0 if step == 1 else 2
                d = mc["d"][0:np_, 1 : 1 + W]
                df = mc["d"][0:np_, lo : lo + W]
                dA = mc["dA"][0:np_, 1 : 1 + W]
                dfA = mc["dA"][0:np_, lo : lo + W]
                c1 = mkpool.tile([np_, W], F16, tag="c1")
                mx = mkpool.tile([np_, W], F32, tag="mx")
                c23 = mkpool.tile([np_, W], F16, tag="c23")
                eng = nc.gpsimd if CMP_ENG == "pool" else nc.vector
                eng.tensor_tensor(out=c1[:], in0=df, in1=d, op=al.is_ge)
                # (dA < d) & (dfA < d)  ==  max(dA, dfA) < d
                eng.tensor_tensor(out=mx[:], in0=dA, in1=dfA, op=al.max)
                eng.tensor_tensor(out=c23[:], in0=mx[:], in1=d, op=al.is_lt)
                mc.update(c1=c1, c23=c23)

            def masks_chain(st, step, mc):
                """Phase 3: AND-tree -> a16; b16 = roll(a,-1,H)."""
                sd = st.sd
                np_ = sd["np"]
                e = (st.wb if step == 1 else st.w1b)[0:np_, 0:W]
                # e3 via tensor_scalar: 4x DVE mode (all-fp16 SBUF)
                e3 = mkpool.tile([np_, W], F16, tag="e3")
                nc.vector.tensor_scalar(out=e3[:], in0=e, scalar1=ELEM_ID,
                                        scalar2=None, op0=al.is_equal)
                ceng = nc.gpsimd if CHAIN_ENG == "pool" else nc.vector
                c123 = mkpool.tile([np_, W], F16, tag="c123")
                ceng.tensor_tensor(out=c123[:], in0=mc["c1"][:],
                                   in1=mc["c23"][:], op=al.logical_and)
                mp = mkpool.tile([np_, 2 * W], F16, tag="mp")
                a16 = mp[:, 0:W]
                ceng.tensor_tensor(out=a16, in0=c123[:], in1=e3[:],
                                   op=al.logical_and)
                b16 = mkpool.tile([np_, W], F16, tag="b16")
                if sd.get("merged"):
                    # block-structured roll via matmul (zeroes boundaries)
                    psx = pmain.tile([np_, 2 * W], F32, tag="ps")
                    nc.tensor.matmul(out=psx[:, W : W + W], lhsT=sdm16[:],
                                     rhs=a16, start=True, stop=True)
                    nc.scalar.copy(b16[:], psx[:, W : W + W])
                else:
                    # b16[p] = a16[p+1]; p=127 zeroed
                    nc.sync.dma_start(out=b16[0:127, :], in_=mp[1:128, 0:W])
                    nc.sync.dma_start(out=b16[127:128, :], in_=z16[0:1, :])
                mc.update(a16=a16, b16=b16, mp=mp)

            def masks_fin(st, step, mc):
                """Phase 4: r16 -> m016 (after the b16 shift has landed)."""
                np_ = st.sd["np"]
                a16, b16 = mc["a16"], mc["b16"]
                r16 = mkpool.tile([np_, W], F16, tag="r16")
                reng = nc.gpsimd if CHAIN_ENG == "pool" else nc.vector
                reng.tensor_tensor(out=r16[:], in0=a16, in1=b16[:],
                                   op=al.logical_or)
                m016 = mc["mp"][:, W : 2 * W]
                nc.vector.tensor_scalar(out=m016, in0=r16[:], scalar1=1.0,
                                        scalar2=None, op0=al.is_lt)
                if step == 1:
                    st.m1 = (mc["mp"], b16)
                else:
                    st.m2 = (mc["mp"], b16)

            def exact1(st):
                """Exact fp32 step-1 density (feeds step-2 masks), via
                pure DVE multiply/adds; result W-padded in w1dp."""
                np_ = st.sd["np"]
                d = st.d32[0:np_, 1 : 1 + W]
                dA = st.dA1[0:np_, 1 : 1 + W]
                dB = st.dB1[0:np_, 0:W]
                mp, b16 = st.m1
                a16 = mp[:, 0:W]
                m016 = mp[:, W : 2 * W]
                t1 = mkpool.tile([np_, W], F32, tag="t1")
                t2 = mkpool.tile([np_, W], F32, tag="t2")
                t3 = mkpool.tile([np_, W], F32, tag="t3")
                meng = nc.gpsimd if EXACT_MULT_ENG == "pool" else nc.vector
                meng.tensor_tensor(out=t1[:], in0=a16, in1=dA, op=al.mult)
                meng.tensor_tensor(out=t2[:], in0=b16[:], in1=dB,
                                   op=al.mult)
                meng.tensor_tensor(out=t3[:], in0=m016, in1=d, op=al.mult)
                w1dp = w1pool.tile([128, WP], F32, tag="w1dp")
                s12 = mkpool.tile([np_, W], F32, tag="s12")
                aeng = nc.gpsimd if EXACT_ADD_ENG == "pool" else nc.vector
                aeng.tensor_tensor(out=s12[:], in0=t1[:], in1=t2[:],
                                   op=al.add)
                aeng.tensor_tensor(out=w1dp[0:np_, 1 : 1 + W],
                                   in0=s12[:], in1=t3[:], op=al.add)
                # circular W pads: col0 <- col512 (w=511), col513 <- col1
                nc.scalar.copy(w1dp[0:np_, 0:1], w1dp[0:np_, W : W + 1])
                nc.scalar.copy(w1dp[0:np_, W + 1 : W + 2],
                               w1dp[0:np_, 1:2])
                st.w1dp = w1dp

            def emit_group(st, step, g):
                """One fp16 stencil group-step: products, matmuls, evac."""
                sd = st.sd
                np_ = sd["np"]
                mp, b16 = st.m1 if step == 1 else st.m2
                src = (st.wb if step == 1 else st.w1b)[
                    0:np_, g * NCH * W : (g + 1) * NCH * W]
                src_v = src.rearrange("p (c w) -> p c w", c=NCH)
                b_b = b16[:].unsqueeze(1).broadcast_to([np_, NCH, W])
                fd = NCH * W
                # Q first: the su-chain consumes it before P/R are needed
                Q = pqrpool.tile([np_, fd], F16, tag="Q")
                Qv = Q[:].rearrange("p (c w) -> p c w", c=NCH)
                h = NCH // 2
                ndve = QMODE(sd["idx"], step, g)
                if ndve < NCH:
                    nc.gpsimd.tensor_tensor(out=Qv[:, ndve:NCH],
                                            in0=b_b[:, ndve:NCH],
                                            in1=src_v[:, ndve:NCH],
                                            op=al.mult)
                if ndve > 0:
                    nc.vector.tensor_tensor(out=Qv[:, 0:ndve],
                                            in0=b_b[:, 0:ndve],
                                            in1=src_v[:, 0:ndve],
                                            op=al.mult)
                # P and R as ONE double-wide op over the packed [a16|m016]
                # mask pair broadcast over channels.
                PR = pqrpool.tile([np_, 2 * fd], F16, tag="PR")
                PRv = PR[:].rearrange("p (k c w) -> p k c w", k=2, c=NCH)
                mp_b = mp.rearrange("p (k w) -> p k w", k=2).unsqueeze(
                    2).broadcast_to([np_, 2, NCH, W])
                src_b = src_v.unsqueeze(1).broadcast_to([np_, 2, NCH, W])
                nc.vector.tensor_tensor(out=PRv, in0=mp_b, in1=src_b,
                                        op=al.mult)
                P = PR[:, 0:fd]
                R = PR[:, fd : 2 * fd]
                if step == 1:
                    og = None
                    dst = st.w1b[0:np_, g * fd : (g + 1) * fd]
                else:
                    og = ogpool.tile([np_, fd], F16, tag="og")
                    dst = og[0:np_, :]
                rmode = RMODE(sd["idx"], step, g)
                if rmode == "dve":
                    ev = ogpool.tile([np_, fd], F16, tag="ev")
                    evac_dst = ev[0:np_, :]
                else:
                    ev = None
                    evac_dst = dst
                hw = 2 * W
                for (c0, c1) in ((0, 1), (2, 3)):
                    ps = pmain.tile([np_, hw], F32, tag="ps")
                    for c in (c0, c1):
                        r = slice((c - c0) * W, (c - c0 + 1) * W)
                        nc.tensor.matmul(out=ps[:, r], lhsT=sd["su16"],
                                         rhs=Q[:, c * W : (c + 1) * W],
                                         start=True, stop=False)
                        last = rmode == "pe"
                        nc.tensor.matmul(out=ps[:, r], lhsT=sd["sd16"],
                                         rhs=P[:, c * W : (c + 1) * W],
                                         start=False, stop=not last)
                        if last:
                            nc.tensor.matmul(out=ps[:, r], lhsT=sd["i16"],
                                             rhs=R[:, c * W : (c + 1) * W],
                                             start=False, stop=True)
                    nc.scalar.copy(evac_dst[:, c0 * W : (c1 + 1) * W], ps[:])
                if rmode == "dve":
                    # disjoint merge: R nonzero only where the chains are 0
                    nc.vector.tensor_tensor(out=dst, in0=ev[0:np_, :],
                                            in1=R, op=al.add)
                if og is not None:
                    sd["store"](og, g)

            def prep_A(sd):
                """Next-set loads: world (fp16) + density tiles (fp32)."""
                st = SetCtx(sd)
                st.wb = wbpool.tile([128, C * W], F16, tag="wb")
                st.d32 = dpool.tile([128, WP], F32, tag="d32")
                st.dA1 = dpool.tile([128, WP], F32, tag="dA1")
                st.dB1 = dpool.tile([128, W], F32, tag="dB1")
                sd["load"](st)
                return st

            def make_main_set(bi, si):
                r_out = si * MAIN_OUT

                def load(st):
                    # mask inputs first: they gate the next set's pipeline
                    _load_rows_2d(nc, st.d32, dpad[bi], r_out - 2, 128)
                    _load_rows_2d(nc, st.dA1, dpad[bi], r_out - 3, 128)
                    _load_rows(nc, st.wb, wd16[bi], r_out - 2, 128, nch=C)
                    _load_rows_2d(nc, st.dB1, dpad[bi], r_out - 1, 128,
                                  c0=1, c1=1 + W)

                def store(og, g):
                    dst = od[bi, g * NCH : (g + 1) * NCH,
                             r_out : r_out + MAIN_OUT, :]
                    nc.scalar.dma_start(
                        out=dst.rearrange("c h w -> h c w"),
                        in_=og[2 : 2 + MAIN_OUT, :].rearrange(
                            "h (c w) -> h c w", c=NCH))

                return {"np": 128, "idx": 4 * bi + si, "su16": su16[:],
                        "sd16": sd16[:], "i16": i16[:], "load": load,
                        "store": store}

            def make_merged_set():
                r_out = 4 * MAIN_OUT
                n_out = H - r_out        # 16

                def load(st):
                    # host pre-stages the merged-set tiles (blocks at the
                    # right partition offsets, zero gaps): 4 clean DMAs,
                    # no Pool memsets (whose cost scales with free size)
                    nc.sync.dma_start(out=st.d32[0:M_NP, :], in_=dm32_d)
                    nc.sync.dma_start(out=st.dA1[0:M_NP, :], in_=dAm32_d)
                    nc.sync.dma_start(
                        out=st.wb[0:M_NP, :].rearrange(
                            "p (c w) -> p c w", c=C), in_=wm16_d)
                    nc.sync.dma_start(out=st.dB1[0:M_NP, 0:W], in_=dBm32_d)

                def store(og, g):
                    for bi, p0 in ((0, 2), (1, M_B1 + 2)):
                        dst = od[bi, g * NCH : (g + 1) * NCH,
                                 r_out : r_out + n_out, :]
                        nc.scalar.dma_start(
                            out=dst.rearrange("c h w -> h c w"),
                            in_=og[p0 : p0 + n_out, :].rearrange(
                                "h (c w) -> h c w", c=NCH))

                return {"np": M_NP, "idx": N_SETS - 1, "su16": sum16[:],
                        "sd16": sdm16[:], "i16": i16[0:M_NP, 0:M_NP],
                        "merged": True, "load": load, "store": store}

            sets = [make_main_set(bi, si)
                    for bi in range(B_PER_CORE) for si in range(4)]
            sets.append(make_merged_set())

            # deep software pipeline. Per-engine queues run in emission
            # order, so long-latency chains (partition-shift DMAs, evac-
            # dependent compares) are split into phases and interleaved
            # between bulk product groups that hide their latency.
            st = prep_A(sets[0])
            load_consts()
            mc1 = masks_rolls(st, 1)
            masks_cmps(st, 1, mc1)
            masks_chain(st, 1, mc1)
            masks_fin(st, 1, mc1)
            exact1(st)
            st.w1b = w1pool.tile([128, C * W], F16, tag="w1b")
            for i in range(len(sets)):
                # phase B: step 1 with step-2 mask phases interleaved
                nxt = sets[i + 1] if i + 1 < len(sets) else None
                emit_group(st, 1, 0)
                mc2 = masks_rolls(st, 2)
                masks_cmps(st, 2, mc2)
                emit_group(st, 1, 1)
                masks_chain(st, 2, mc2)
                emit_group(st, 1, 2)
                masks_fin(st, 2, mc2)
                stn = prep_A(nxt) if nxt else None
                emit_group(st, 1, 3)
                if stn:
                    mc1 = masks_rolls(stn, 1)
                    masks_cmps(stn, 1, mc1)
                emit_group(st, 1, 4)
                # phase C: step 2 with next-set mask chain interleaved
                emit_group(st, 2, 0)
                if stn:
                    masks_chain(stn, 1, mc1)
                emit_group(st, 2, 1)
                if stn:
                    masks_fin(stn, 1, mc1)
                emit_group(st, 2, 2)
                if stn:
                    exact1(stn)
                    stn.w1b = w1pool.tile([128, C * W], F16, tag="w1b")
                emit_group(st, 2, 3)
                emit_group(st, 2, 4)
                st = stn

    nc.compile()
    return nc


def _shift_mats():
    su = np.zeros((128, 128), np.float16)   # out[m] = in[m-1]
    sdn = np.zeros((128, 128), np.float16)  # out[m] = in[m+1]
    for m in range(128):
        if m >= 1:
            su[m - 1, m] = 1.0
        if m <= 126:
            sdn[m + 1, m] = 1.0
    sum_ = np.zeros((M_NP, M_NP), np.float32)
    sdm = np.zeros((M_NP, M_NP), np.float32)
    for base in (0, M_B1):
        for m in range(20):
            if m >= 1:
                sum_[base + m - 1, base + m] = 1.0
            if m <= 18:
                sdm[base + m + 1, base + m] = 1.0
    return su, sdn, sum_, sdm


_NC_CACHE = {}


def kernel(world, rand_movement=None, rand_interact=None, rand_element=None,
           **_ignored):
    world = np.ascontiguousarray(world, dtype=np.float32)
    assert world.shape == (B, C, H, W), world.shape
    if "nc" not in _NC_CACHE:
        _NC_CACHE["nc"] = build_kernel()
    nc = _NC_CACHE["nc"]
    su, sdn, sum_, sdm = _shift_mats()
    i16 = np.eye(128, dtype=np.float16)
    world16 = world.astype(np.float16)
    d = world[:, 1]                                # [B,H,W] fp32 density
    dpad = np.concatenate([d[:, :, W - 1 :], d, d[:, :, :1]], axis=2)
    dpad = np.ascontiguousarray(dpad, dtype=np.float32)
    # host-staged merged-set tiles (last 16 rows of each batch + halo)
    r_out = 4 * MAIN_OUT
    n_rows = H - r_out + 4                         # 20
    in_maps = []
    for core in range(N_CORES):
        sl = slice(core * B_PER_CORE, (core + 1) * B_PER_CORE)
        w16c = world16[sl]
        dpc = dpad[sl]
        wm = np.zeros((M_NP, C, W), np.float16)
        dm = np.zeros((M_NP, WP), np.float32)
        dAm = np.zeros((M_NP, WP), np.float32)
        dBm = np.zeros((M_NP, W), np.float32)
        for bi, p0 in ((0, 0), (1, M_B1)):
            r0 = np.arange(r_out - 2, r_out - 2 + n_rows) % H
            wm[p0 : p0 + n_rows] = w16c[bi].transpose(1, 0, 2)[r0]
            dm[p0 : p0 + n_rows] = dpc[bi][r0]
            dAm[p0 : p0 + n_rows] = dpc[bi][(r0 - 1) % H]
            dBm[p0 : p0 + n_rows] = dpc[bi][(r0 + 1) % H][:, 1 : 1 + W]
        in_maps.append({
            "world16": np.ascontiguousarray(w16c),
            "dpad32": dpc,
            "su16": su, "sd16": sdn, "i16": i16,
            "sum16": sum_.astype(np.float16),
            "sdm16": sdm.astype(np.float16),
            "sum32": sum_,
            "wm16": wm, "dm32": dm, "dAm32": dAm, "dBm32": dBm,
        })
    res = run_bass_kernel_spmd(nc, in_maps, list(range(N_CORES)),
                               trace=_NC_CACHE.get("trace", False))
    _NC_CACHE["last_result"] = res
    out = np.concatenate([r["out16"] for r in res.results], axis=0)
    return out.astype(np.float32)


if __name__ == "__main__":
    rng = np.random.default_rng(0)
    w = rng.standard_normal((B, C, H, W)).astype(np.float32)
    w[:, 0] = rng.integers(0, 10, (B, H, W)).astype(np.float32)
    out = kernel(w)
    print("ran:", out.shape, out.dtype)


# revision 29
# speedup vs baseline: 1.3008x; 1.3008x over previous
"""Trainium2 Bass kernel for BehaviorLemming, v3.

Two fused stencil steps, data-parallel over batch (B=16 / 8 cores).
Layout: H rows in partitions, (channel, W) in free dim, 5 groups of 4ch.

v3 vs baseline:
- World movement in fp16: products P=a*w, Q=b*w, R=m0*w as fp16 tiles;
  row shifts as fp16 matmuls (1 cyc/row vs fp32's 4). The stay term R
  rides a third identity-matmul chain into PSUM, so the copy_predicated
  pass disappears; PSUM = su@Q + sd@P + I@R is the complete output.
- Exact fp32 side-path for step-1 density (ch1) only: step-2 mask
  comparisons must see bit-exact step-1 densities. Final outputs
  tolerate fp16 rounding (gate 2e-2, fp16 gives ~5e-4).
- Mask row-shifts (b = roll(a,-1), dA2 = roll(d2,+1)) as tiny matmuls;
  density-above (dA1) loaded straight from HBM at a row offset; mask
  W-rolls folded into shifted free-axis APs of the compare ops.
- Stores in fp16 (host converts): ~half the store traffic.
- P and R emitted as one double-wide DVE op over a packed [a16|m016]
  mask pair; Q split 2ch Pool / 2ch DVE; conversions + PSUM
  evacuations on Act. Deep software pipeline: per-engine queues run in
  emission order, so mask phases and next-set prep are interleaved
  between product groups that hide their latency.
"""

import numpy as np

import concourse.bacc as bacc
import concourse.mybir as mybir
import concourse.tile as tile
from concourse.bass_utils import run_bass_kernel_spmd

B, C, H, W = 16, 20, 512, 512
N_CORES = 8
B_PER_CORE = B // N_CORES
ELEM_ID = 3.0
F32 = mybir.dt.float32
F16 = mybir.dt.float16
NCH = 4
NGRP = C // NCH
MAIN_OUT = 124
M_B1 = 32              # partition offset of batch-1 block in the merged set
M_NP = 52

# knob per (step, group): where the Q product runs.
# "pool" = all 4ch on Pool, "split" = 2ch Pool + 2ch DVE, "dve" = all DVE
QMODE = {}
for _s in (1, 2):
    for _g in range(5):
        QMODE[(_s, _g)] = "split"
QMODE[(2, 4)] = "pool"

# which groups' conv (fp32->fp16 world copy) run on Act (rest DVE)
CONV_ACT = {0, 1, 2, 3, 4}


def _load_rows(nc, dst_tile, src_ap, row_start, n_rows, p0=0, nch=NCH):
    """Load n_rows (mod H, split at wrap) of src [nch,H,W] into dst
    partitions [p0, p0+n_rows), free dim = (c, w)."""
    s = row_start % H
    remaining = n_rows
    while remaining > 0:
        n = min(remaining, H - s)
        src = src_ap[:, s : s + n, :].rearrange("c h w -> h c w")
        nc.sync.dma_start(out=dst_tile[p0 : p0 + n, :].rearrange(
            "h (c w) -> h c w", c=nch), in_=src)
        p0 += n
        s = (s + n) % H
        remaining -= n


def _cmp_rolled(nc, al, out, rolled_src, base, shift_w, op):
    """out = op(roll(rolled_src, shift_w, W), base), via shifted free-axis
    APs: no materialized roll. Two pieces (bulk + 1-col wrap)."""
    if shift_w == 1:
        nc.vector.tensor_tensor(out=out[:, 1:W], in0=rolled_src[:, 0 : W - 1],
                                in1=base[:, 1:W], op=op)
        nc.vector.tensor_tensor(out=out[:, 0:1], in0=rolled_src[:, W - 1 : W],
                                in1=base[:, 0:1], op=op)
    else:
        nc.vector.tensor_tensor(out=out[:, 0 : W - 1], in0=rolled_src[:, 1:W],
                                in1=base[:, 0 : W - 1], op=op)
        nc.vector.tensor_tensor(out=out[:, W - 1 : W], in0=rolled_src[:, 0:1],
                                in1=base[:, W - 1 : W], op=op)


class SetCtx:
    """Per-set emission state."""

    def __init__(self, sd):
        self.sd = sd
        self.wb = None        # [np,10240] f16 world
        self.g0 = None        # [np,2048] f32 (ch0..3) for masks + exact
        self.dA1 = None       # [np,512] f32 density rolled +1 (HBM load)
        self.w1b = None       # [np,10240] f16 step-1 world
        self.w1d = None       # [np,512] f32 exact step-1 density
        self.m1 = None        # (a16, b16, m016) step-1
        self.m2 = None


def build_kernel():
    nc = bacc.Bacc("TRN2", target_bir_lowering=False, debug=False,
                   num_devices=N_CORES)
    wd = nc.dram_tensor("world", [B_PER_CORE, C, H, W], F32,
                        kind="ExternalInput").ap()
    su32_d = nc.dram_tensor("su32", [128, 128], F32, kind="ExternalInput").ap()
    sd32_d = nc.dram_tensor("sd32", [128, 128], F32, kind="ExternalInput").ap()
    su16_d = nc.dram_tensor("su16", [128, 128], F16, kind="ExternalInput").ap()
    sd16_d = nc.dram_tensor("sd16", [128, 128], F16, kind="ExternalInput").ap()
    i16_d = nc.dram_tensor("i16", [128, 128], F16, kind="ExternalInput").ap()
    i32_d = nc.dram_tensor("i32", [128, 128], F32, kind="ExternalInput").ap()
    sum32_d = nc.dram_tensor("sum32", [M_NP, M_NP], F32,
                             kind="ExternalInput").ap()
    sdm32_d = nc.dram_tensor("sdm32", [M_NP, M_NP], F32,
                             kind="ExternalInput").ap()
    sum16_d = nc.dram_tensor("sum16", [M_NP, M_NP], F16,
                             kind="ExternalInput").ap()
    sdm16_d = nc.dram_tensor("sdm16", [M_NP, M_NP], F16,
                             kind="ExternalInput").ap()
    od = nc.dram_tensor("out16", [B_PER_CORE, C, H, W], F16,
                        kind="ExternalOutput").ap()

    al = mybir.AluOpType

    with tile.TileContext(nc) as tc:
        with (
            tc.tile_pool(name="const", bufs=1) as cpool,
            tc.tile_pool(name="stg", bufs=2) as stgpool,
            tc.tile_pool(name="g0p", bufs=2) as g0pool,
            tc.tile_pool(name="wbp", bufs=2) as wbpool,
            tc.tile_pool(name="w1p", bufs=2) as w1pool,
            tc.tile_pool(name="mkp", bufs=2) as mkpool,
            tc.tile_pool(name="pqr", bufs=3) as pqrpool,
            tc.tile_pool(name="ogp", bufs=2) as ogpool,
            tc.tile_pool(name="pmain", bufs=4, space="PSUM") as pmain,
        ):
            su32 = cpool.tile([128, 128], F32)
            sd32 = cpool.tile([128, 128], F32)
            su16 = cpool.tile([128, 128], F16)
            sd16 = cpool.tile([128, 128], F16)
            i16 = cpool.tile([128, 128], F16)
            i32 = cpool.tile([128, 128], F32)
            sum32 = cpool.tile([M_NP, M_NP], F32)
            sdm32 = cpool.tile([M_NP, M_NP], F32)
            sum16 = cpool.tile([M_NP, M_NP], F16)
            sdm16 = cpool.tile([M_NP, M_NP], F16)
            def load_consts():
                # Act-queue HWDGE: runs in parallel with the set-0 world
                # loads on SP instead of queueing behind them
                for t, d in ((su32, su32_d), (sd32, sd32_d), (su16, su16_d),
                             (sd16, sd16_d), (i16, i16_d), (i32, i32_d),
                             (sum32, sum32_d),
                             (sdm32, sdm32_d), (sum16, sum16_d),
                             (sdm16, sdm16_d)):
                    nc.scalar.dma_start(out=t[:], in_=d)
            ones16 = cpool.tile([128, W], F16)
            nc.gpsimd.memset(ones16[:], 1.0)

            def masks_rolls(st, step):
                """Phase 1: dA2 = roll(d2,+1,H) via fp32 matmul (step 2)."""
                sd = st.sd
                np_ = sd["np"]
                if step == 1:
                    d = st.g0[0:np_, W : 2 * W]
                    dA = st.dA1[0:np_, :]
                    return {"d": d, "dA": dA}
                d = st.w1d[0:np_, :]
                psx = pmain.tile([np_, 2 * W], F32, tag="ps")
                nc.tensor.matmul(out=psx[:, 0:W], lhsT=sd["su32"], rhs=d,
                                 start=True, stop=True)
                dAt = mkpool.tile([np_, W], F32, tag="dA2")
                nc.scalar.copy(dAt[:], psx[:, 0:W])
                return {"d": d, "dA": dAt[:], "psx": psx}

            def masks_cmps(st, step, mc):
                """Phase 2: density comparisons via shifted free-axis APs."""
                np_ = st.sd["np"]
                shift_w = 1 if step == 1 else -1
                d, dA = mc["d"], mc["dA"]
                c1 = mkpool.tile([np_, W], F16, tag="c1")
                c2 = mkpool.tile([np_, W], F16, tag="c2")
                c3 = mkpool.tile([np_, W], F16, tag="c3")
                _cmp_rolled(nc, al, c1, d, d, shift_w, al.is_ge)
                nc.vector.tensor_tensor(out=c2[:], in0=dA, in1=d, op=al.is_lt)
                _cmp_rolled(nc, al, c3, dA, d, shift_w, al.is_lt)
                mc.update(c1=c1, c2=c2, c3=c3)

            def masks_chain(st, step, mc):
                """Phase 3: AND-tree -> a16; b16 = roll(a,-1,H) via matmul."""
                sd = st.sd
                np_ = sd["np"]
                e = st.g0[0:np_, 0:W] if step == 1 else st.w1b[0:np_, 0:W]
                e3c3 = mkpool.tile([np_, W], F16, tag="e3")
                nc.vector.scalar_tensor_tensor(out=e3c3[:], in0=e,
                                               scalar=ELEM_ID,
                                               in1=mc["c3"][:],
                                               op0=al.is_equal,
                                               op1=al.logical_and)
                c12 = mkpool.tile([np_, W], F16, tag="c12")
                nc.vector.tensor_tensor(out=c12[:], in0=mc["c1"][:],
                                        in1=mc["c2"][:], op=al.logical_and)
                mp = mkpool.tile([np_, 2 * W], F16, tag="mp")
                a16 = mp[:, 0:W]
                nc.vector.tensor_tensor(out=a16, in0=c12[:], in1=e3c3[:],
                                        op=al.logical_and)
                # b16[m] = a16[m+1]; sd16 zeroes the boundary rows natively
                psx = mc.get("psx")
                if psx is None:
                    psx = pmain.tile([np_, 2 * W], F32, tag="ps")
                    mc["psx"] = psx
                nc.tensor.matmul(out=psx[:, W : 2 * W], lhsT=sd["sd16"],
                                 rhs=a16, start=True, stop=True)
                b16 = mkpool.tile([np_, W], F16, tag="b16")
                nc.scalar.copy(b16[:], psx[:, W : 2 * W])
                mc.update(a16=a16, b16=b16, mp=mp)

            def masks_fin(st, step, mc):
                """Phase 4: r16/m016 (after the b16 DMA has had time)."""
                np_ = st.sd["np"]
                a16, b16 = mc["a16"], mc["b16"]
                r16 = mkpool.tile([np_, W], F16, tag="r16")
                nc.vector.tensor_tensor(out=r16[:], in0=a16, in1=b16[:],
                                        op=al.logical_or)
                m016 = mc["mp"][:, W : 2 * W]
                # r < 1 == (r == 0) for 0/1 masks; all-f16 operands -> 2x DVE
                nc.vector.tensor_tensor(out=m016, in0=r16[:],
                                        in1=ones16[0:np_, :], op=al.is_lt)
                if step == 1:
                    st.m1 = (mc["mp"], b16)
                else:
                    st.m2 = (mc["mp"], b16)

            def exact1(st, mc):
                """Exact fp32 density path (feeds step-2 comparisons)."""
                sd = st.sd
                np_ = sd["np"]
                d = st.g0[0:np_, W : 2 * W]
                mp, b16 = st.m1
                a16 = mp[:, 0:W]
                m016 = mp[:, W : 2 * W]
                P0 = mkpool.tile([np_, W], F32, tag="P0")
                Q0 = mkpool.tile([np_, W], F32, tag="Q0")
                R0 = mkpool.tile([np_, W], F32, tag="R0")
                nc.vector.tensor_tensor(out=P0[:], in0=a16, in1=d,
                                        op=al.mult)
                nc.vector.tensor_tensor(out=Q0[:], in0=b16[:], in1=d,
                                        op=al.mult)
                nc.vector.tensor_tensor(out=R0[:], in0=m016, in1=d,
                                        op=al.mult)
                psd = pmain.tile([np_, 2 * W], F32, tag="ps")
                nc.tensor.matmul(out=psd[:, 0:W], lhsT=sd["su32"],
                                 rhs=Q0[:], start=True, stop=False)
                nc.tensor.matmul(out=psd[:, 0:W], lhsT=sd["sd32"],
                                 rhs=P0[:], start=False, stop=True)
                w1d = w1pool.tile([np_, W], F32, tag="w1d")
                nc.vector.tensor_tensor(out=w1d[:], in0=psd[:, 0:W],
                                        in1=R0[:], op=al.add)
                st.w1d = w1d

            def emit_group(st, step, g):
                """One fp16 stencil group-step: products, matmuls, evac."""
                sd = st.sd
                np_ = sd["np"]
                mp, b16 = st.m1 if step == 1 else st.m2
                src = (st.wb if step == 1 else st.w1b)[
                    0:np_, g * NCH * W : (g + 1) * NCH * W]
                src_v = src.rearrange("p (c w) -> p c w", c=NCH)
                b_b = b16[:].unsqueeze(1).broadcast_to([np_, NCH, W])
                fd = NCH * W
                mode = QMODE[(step, g)]
                if sd.get("last") and step == 2 and g >= 3:
                    mode = "dve"    # shorten the drain tail
                # Q first: the su-chain consumes it before P/R are needed
                Q = pqrpool.tile([np_, fd], F16, tag="Q")
                Qv = Q[:].rearrange("p (c w) -> p c w", c=NCH)
                h = NCH // 2
                if mode in ("split", "rpool"):
                    nc.gpsimd.tensor_tensor(
                        out=Qv[:, 0:h], in0=b_b[:, 0:h], in1=src_v[:, 0:h],
                        op=al.mult)
                    nc.vector.tensor_tensor(
                        out=Qv[:, h:NCH], in0=b_b[:, h:NCH],
                        in1=src_v[:, h:NCH], op=al.mult)
                    halves = ((2, 3), (0, 1))   # DVE-made half first
                elif mode == "pool":
                    # two half-ops so the first half's matmuls start sooner
                    nc.gpsimd.tensor_tensor(
                        out=Qv[:, 0:h], in0=b_b[:, 0:h], in1=src_v[:, 0:h],
                        op=al.mult)
                    nc.gpsimd.tensor_tensor(
                        out=Qv[:, h:NCH], in0=b_b[:, h:NCH],
                        in1=src_v[:, h:NCH], op=al.mult)
                    halves = ((0, 1), (2, 3))
                else:
                    nc.vector.tensor_tensor(out=Qv, in0=b_b, in1=src_v,
                                            op=al.mult)
                    halves = ((0, 1), (2, 3))
                PR = pqrpool.tile([np_, 2 * fd], F16, tag="PR")
                if mode == "rpool":
                    # P on DVE; R fully on Pool (I-chain consumes R last,
                    # so Pool's latency is tolerable)
                    a_b = mp[:, 0:W].unsqueeze(1).broadcast_to(
                        [np_, NCH, W])
                    m_b = mp[:, W : 2 * W].unsqueeze(1).broadcast_to(
                        [np_, NCH, W])
                    nc.vector.tensor_tensor(
                        out=PR[:, 0:fd].rearrange("p (c w) -> p c w", c=NCH),
                        in0=a_b, in1=src_v, op=al.mult)
                    nc.gpsimd.tensor_tensor(
                        out=PR[:, fd : 2 * fd].rearrange(
                            "p (c w) -> p c w", c=NCH),
                        in0=m_b, in1=src_v, op=al.mult)
                else:
                    # P and R as ONE double-wide DVE op: out [np,2,NCH,W],
                    # masks [a16 | m016] broadcast over channels, src
                    # broadcast over the P/R axis.
                    PRv = PR[:].rearrange("p (k c w) -> p k c w", k=2, c=NCH)
                    mp_b = mp.rearrange("p (k w) -> p k w", k=2).unsqueeze(
                        2).broadcast_to([np_, 2, NCH, W])
                    src_b = src_v.unsqueeze(1).broadcast_to([np_, 2, NCH, W])
                    nc.vector.tensor_tensor(out=PRv, in0=mp_b, in1=src_b,
                                            op=al.mult)
                P = PR[:, 0:fd]
                R = PR[:, fd : 2 * fd]
                if step == 1:
                    og = None
                    dst = st.w1b[0:np_, g * fd : (g + 1) * fd]
                else:
                    og = ogpool.tile([np_, fd], F16, tag="og")
                    dst = og[0:np_, :]
                hw = 2 * W
                for (c0, c1) in halves:
                    ps = pmain.tile([np_, hw], F32, tag="ps")
                    for c in (c0, c1):
                        r = slice((c - c0) * W, (c - c0 + 1) * W)
                        nc.tensor.matmul(out=ps[:, r], lhsT=sd["su16"],
                                         rhs=Q[:, c * W : (c + 1) * W],
                                         start=True, stop=False)
                        nc.tensor.matmul(out=ps[:, r], lhsT=sd["sd16"],
                                         rhs=P[:, c * W : (c + 1) * W],
                                         start=False, stop=False)
                        nc.tensor.matmul(out=ps[:, r], lhsT=sd["i16"],
                                         rhs=R[:, c * W : (c + 1) * W],
                                         start=False, stop=True)
                    nc.scalar.copy(dst[:, c0 * W : (c1 + 1) * W], ps[:])
                if og is not None:
                    sd["store"](og, g)

            def prep_A(sd):
                """Next-set loads for g0 + dA1, conv g0."""
                st = SetCtx(sd)
                np_ = sd["np"]
                st.g0 = g0pool.tile([128, NCH * W], F32, tag="g0")
                sd["load"](st.g0, 0)
                st.dA1 = g0pool.tile([128, W], F32, tag="dA1")
                sd["load_dA1"](st.dA1)
                st.wb = wbpool.tile([128, C * W], F16, tag="wb")
                nc.scalar.copy(st.wb[0:np_, 0 : NCH * W], st.g0[0:np_, :])
                return st

            def prep_B(st, mix_dve=False):
                """Remaining group loads + conversions. mix_dve spreads
                conversions over DVE too (startup, when DVE is idle)."""
                np_ = st.sd["np"]
                for g in range(1, NGRP):
                    stg = stgpool.tile([128, NCH * W], F32, tag="stg")
                    st.sd["load"](stg, g)
                    dst = st.wb[0:np_, g * NCH * W : (g + 1) * NCH * W]
                    if (g in CONV_ACT) and not (mix_dve and g % 2):
                        nc.scalar.copy(dst, stg[0:np_, :])
                    else:
                        nc.vector.tensor_copy(dst, stg[0:np_, :])

            def make_main_set(bi, si):
                r_out = si * MAIN_OUT

                def load(t, g):
                    _load_rows(nc, t, wd[bi, g * NCH : (g + 1) * NCH],
                               r_out - 2, 128)

                def load_dA1(t):
                    _load_rows(nc, t, wd[bi, 1:2], r_out - 3, 128, nch=1)

                def store(og, g):
                    dst = od[bi, g * NCH : (g + 1) * NCH,
                             r_out : r_out + MAIN_OUT, :]
                    nc.scalar.dma_start(
                        out=dst.rearrange("c h w -> h c w"),
                        in_=og[2 : 2 + MAIN_OUT, :].rearrange(
                            "h (c w) -> h c w", c=NCH))

                return {"np": 128, "su32": su32[:], "sd32": sd32[:],
                        "su16": su16[:], "sd16": sd16[:],
                        "i16": i16[:], "i32": i32[:], "blocks": [(0, 128)],
                        "load": load, "load_dA1": load_dA1, "store": store}

            def make_merged_set():
                r_out = 4 * MAIN_OUT
                n_out = H - r_out        # 16
                blocks = [(0, n_out + 4), (M_B1, n_out + 4)]

                def load(t, g):
                    nc.gpsimd.memset(t[0:64, :], 0.0)
                    for bi, p0 in ((0, 0), (1, M_B1)):
                        _load_rows(nc, t, wd[bi, g * NCH : (g + 1) * NCH],
                                   r_out - 2, n_out + 4, p0=p0)

                def load_dA1(t):
                    nc.gpsimd.memset(t[0:64, :], 0.0)
                    for bi, p0 in ((0, 0), (1, M_B1)):
                        _load_rows(nc, t, wd[bi, 1:2], r_out - 3, n_out + 4,
                                   p0=p0, nch=1)

                def store(og, g):
                    for bi, p0 in ((0, 2), (1, M_B1 + 2)):
                        dst = od[bi, g * NCH : (g + 1) * NCH,
                                 r_out : r_out + n_out, :]
                        nc.scalar.dma_start(
                            out=dst.rearrange("c h w -> h c w"),
                            in_=og[p0 : p0 + n_out, :].rearrange(
                                "h (c w) -> h c w", c=NCH))

                return {"np": M_NP, "su32": sum32[:], "sd32": sdm32[:],
                        "su16": sum16[:], "sd16": sdm16[:],
                        "i16": i16[0:M_NP, 0:M_NP],
                        "i32": i32[0:M_NP, 0:M_NP], "blocks": blocks,
                        "last": True,
                        "load": load, "load_dA1": load_dA1, "store": store}

            sets = [make_main_set(bi, si)
                    for bi in range(B_PER_CORE) for si in range(4)]
            sets.append(make_merged_set())

            # deep software pipeline. Per-engine queues run in emission
            # order, so long-latency chains (partition-shift DMAs, evac-
            # dependent compares) are split into phases and interleaved
            # between bulk product groups that hide their latency.
            st = prep_A(sets[0])
            prep_B(st, mix_dve=True)
            load_consts()
            mc1 = masks_rolls(st, 1)
            masks_cmps(st, 1, mc1)
            masks_chain(st, 1, mc1)
            masks_fin(st, 1, mc1)
            exact1(st, mc1)
            st.w1b = w1pool.tile([128, C * W], F16, tag="w1b")
            for i in range(len(sets)):
                # phase B: step 1 with step-2 mask phases interleaved
                emit_group(st, 1, 0)
                mc2 = masks_rolls(st, 2)
                masks_cmps(st, 2, mc2)
                emit_group(st, 1, 1)
                masks_chain(st, 2, mc2)
                emit_group(st, 1, 2)
                masks_fin(st, 2, mc2)
                emit_group(st, 1, 3)
                emit_group(st, 1, 4)
                # phase C: step 2 with next-set prep interleaved
                nxt = sets[i + 1] if i + 1 < len(sets) else None
                stn = prep_A(nxt) if nxt else None
                emit_group(st, 2, 0)
                if stn:
                    prep_B(stn)
                emit_group(st, 2, 1)
                if stn:
                    mc1 = masks_rolls(stn, 1)
                    masks_cmps(stn, 1, mc1)
                emit_group(st, 2, 2)
                if stn:
                    masks_chain(stn, 1, mc1)
                    masks_fin(stn, 1, mc1)
                emit_group(st, 2, 3)
                if stn:
                    exact1(stn, mc1)
                    stn.w1b = w1pool.tile([128, C * W], F16, tag="w1b")
                emit_group(st, 2, 4)
                st = stn

    nc.compile()
    return nc


def _shift_mats():
    su = np.zeros((128, 128), np.float32)   # out[m] = in[m-1]
    sdn = np.zeros((128, 128), np.float32)  # out[m] = in[m+1]
    for m in range(128):
        if m >= 1:
            su[m - 1, m] = 1.0
        if m <= 126:
            sdn[m + 1, m] = 1.0
    sum_ = np.zeros((M_NP, M_NP), np.float32)
    sdm = np.zeros((M_NP, M_NP), np.float32)
    for base in (0, M_B1):
        for m in range(20):
            if m >= 1:
                sum_[base + m - 1, base + m] = 1.0
            if m <= 18:
                sdm[base + m + 1, base + m] = 1.0
    return su, sdn, sum_, sdm


_NC_CACHE = {}


def kernel(world, rand_movement=None, rand_interact=None, rand_element=None,
           **_ignored):
    world = np.ascontiguousarray(world, dtype=np.float32)
    assert world.shape == (B, C, H, W), world.shape
    if "nc" not in _NC_CACHE:
        _NC_CACHE["nc"] = build_kernel()
    nc = _NC_CACHE["nc"]
    su, sdn, sum_, sdm = _shift_mats()
    i16 = np.eye(128, dtype=np.float16)
    in_maps = []
    for core in range(N_CORES):
        shard = world[core * B_PER_CORE : (core + 1) * B_PER_CORE]
        in_maps.append({
            "world": np.ascontiguousarray(shard),
            "su32": su, "sd32": sdn,
            "su16": su.astype(np.float16), "sd16": sdn.astype(np.float16),
            "i16": i16, "i32": np.eye(128, dtype=np.float32),
            "sum32": sum_, "sdm32": sdm,
            "sum16": sum_.astype(np.float16),
            "sdm16": sdm.astype(np.float16),
        })
    res = run_bass_kernel_spmd(nc, in_maps, list(range(N_CORES)),
                               trace=_NC_CACHE.get("trace", False))
    _NC_CACHE["last_result"] = res
    out = np.concatenate([r["out16"] for r in res.results], axis=0)
    return out.astype(np.float32)


if __name__ == "__main__":
    rng = np.random.default_rng(0)
    w = rng.standard_normal((B, C, H, W)).astype(np.float32)
    w[:, 0] = rng.integers(0, 10, (B, H, W)).astype(np.float32)
    out = kernel(w)
    print("ran:", out.shape, out.dtype)


# revision 30
# speedup vs baseline: 1.3155x; 1.0113x over previous
"""Trainium2 Bass kernel for BehaviorLemming, v3.

Two fused stencil steps, data-parallel over batch (B=16 / 8 cores).
Layout: H rows in partitions, (channel, W) in free dim, 5 groups of 4ch.

v3 vs baseline:
- World movement in fp16: products P=a*w, Q=b*w, R=m0*w as fp16 tiles;
  row shifts as fp16 matmuls (1 cyc/row vs fp32's 4). The stay term R
  rides a third identity-matmul chain into PSUM, so the copy_predicated
  pass disappears; PSUM = su@Q + sd@P + I@R is the complete output.
- Exact fp32 side-path for step-1 density (ch1) only: step-2 mask
  comparisons must see bit-exact step-1 densities. Final outputs
  tolerate fp16 rounding (gate 2e-2, fp16 gives ~5e-4).
- Mask row-shifts (b = roll(a,-1), dA2 = roll(d2,+1)) as tiny matmuls;
  density-above (dA1) loaded straight from HBM at a row offset; mask
  W-rolls folded into shifted free-axis APs of the compare ops.
- Stores in fp16 (host converts): ~half the store traffic.
- P and R emitted as one double-wide DVE op over a packed [a16|m016]
  mask pair; Q split 2ch Pool / 2ch DVE; conversions + PSUM
  evacuations on Act. Deep software pipeline: per-engine queues run in
  emission order, so mask phases and next-set prep are interleaved
  between product groups that hide their latency.
"""

import numpy as np

import concourse.bacc as bacc
import concourse.mybir as mybir
import concourse.tile as tile
from concourse.bass_utils import run_bass_kernel_spmd

B, C, H, W = 16, 20, 512, 512
N_CORES = 8
B_PER_CORE = B // N_CORES
ELEM_ID = 3.0
F32 = mybir.dt.float32
F16 = mybir.dt.float16
NCH = 4
NGRP = C // NCH
MAIN_OUT = 124
M_B1 = 32              # partition offset of batch-1 block in the merged set
M_NP = 52

# knob per (step, group): where the Q product runs.
# "pool" = all 4ch on Pool, "split" = 2ch Pool + 2ch DVE, "dve" = all DVE
QMODE = {}
for _s in (1, 2):
    for _g in range(5):
        QMODE[(_s, _g)] = "split"
QMODE[(2, 4)] = "pool"

# which groups' conv (fp32->fp16 world copy) run on Act (rest DVE)
CONV_ACT = {0, 1, 2, 3, 4}


def _load_rows(nc, dst_tile, src_ap, row_start, n_rows, p0=0, nch=NCH):
    """Load n_rows (mod H, split at wrap) of src [nch,H,W] into dst
    partitions [p0, p0+n_rows), free dim = (c, w)."""
    s = row_start % H
    remaining = n_rows
    while remaining > 0:
        n = min(remaining, H - s)
        src = src_ap[:, s : s + n, :].rearrange("c h w -> h c w")
        nc.sync.dma_start(out=dst_tile[p0 : p0 + n, :].rearrange(
            "h (c w) -> h c w", c=nch), in_=src)
        p0 += n
        s = (s + n) % H
        remaining -= n


def _load_rows_2d(nc, dst_tile, src_ap, row_start, n_rows, p0=0):
    """Load n_rows (mod H) of src [H, W] into dst partitions."""
    s = row_start % H
    remaining = n_rows
    while remaining > 0:
        n = min(remaining, H - s)
        nc.sync.dma_start(out=dst_tile[p0 : p0 + n, :],
                          in_=src_ap[s : s + n, :])
        p0 += n
        s = (s + n) % H
        remaining -= n


def _cmp_rolled(nc, al, out, rolled_src, base, shift_w, op):
    """out = op(roll(rolled_src, shift_w, W), base), via shifted free-axis
    APs: no materialized roll. Two pieces (bulk + 1-col wrap)."""
    if shift_w == 1:
        nc.vector.tensor_tensor(out=out[:, 1:W], in0=rolled_src[:, 0 : W - 1],
                                in1=base[:, 1:W], op=op)
        nc.vector.tensor_tensor(out=out[:, 0:1], in0=rolled_src[:, W - 1 : W],
                                in1=base[:, 0:1], op=op)
    else:
        nc.vector.tensor_tensor(out=out[:, 0 : W - 1], in0=rolled_src[:, 1:W],
                                in1=base[:, 0 : W - 1], op=op)
        nc.vector.tensor_tensor(out=out[:, W - 1 : W], in0=rolled_src[:, 0:1],
                                in1=base[:, W - 1 : W], op=op)


class SetCtx:
    """Per-set emission state."""

    def __init__(self, sd):
        self.sd = sd
        self.wb = None        # [np,10240] f16 world
        self.d32 = None       # [np,512] f32 density (ch1) for masks
        self.dA1 = None       # [np,512] f32 density rolled +1 (HBM load)
        self.w1b = None       # [np,10240] f16 step-1 world
        self.w1d = None       # [np,512] f32 exact step-1 density
        self.m1 = None        # (a16, b16, m016) step-1
        self.m2 = None


def build_kernel():
    nc = bacc.Bacc("TRN2", target_bir_lowering=False, debug=False,
                   num_devices=N_CORES)
    wd16 = nc.dram_tensor("world16", [B_PER_CORE, C, H, W], F16,
                          kind="ExternalInput").ap()
    wd32 = nc.dram_tensor("wd32", [B_PER_CORE, H, W], F32,
                          kind="ExternalInput").ap()
    su32_d = nc.dram_tensor("su32", [128, 128], F32, kind="ExternalInput").ap()
    sd32_d = nc.dram_tensor("sd32", [128, 128], F32, kind="ExternalInput").ap()
    su16_d = nc.dram_tensor("su16", [128, 128], F16, kind="ExternalInput").ap()
    sd16_d = nc.dram_tensor("sd16", [128, 128], F16, kind="ExternalInput").ap()
    i16_d = nc.dram_tensor("i16", [128, 128], F16, kind="ExternalInput").ap()
    i32_d = nc.dram_tensor("i32", [128, 128], F32, kind="ExternalInput").ap()
    sum32_d = nc.dram_tensor("sum32", [M_NP, M_NP], F32,
                             kind="ExternalInput").ap()
    sdm32_d = nc.dram_tensor("sdm32", [M_NP, M_NP], F32,
                             kind="ExternalInput").ap()
    sum16_d = nc.dram_tensor("sum16", [M_NP, M_NP], F16,
                             kind="ExternalInput").ap()
    sdm16_d = nc.dram_tensor("sdm16", [M_NP, M_NP], F16,
                             kind="ExternalInput").ap()
    od = nc.dram_tensor("out16", [B_PER_CORE, C, H, W], F16,
                        kind="ExternalOutput").ap()

    al = mybir.AluOpType

    with tile.TileContext(nc) as tc:
        with (
            tc.tile_pool(name="const", bufs=1) as cpool,
            tc.tile_pool(name="stg", bufs=2) as stgpool,
            tc.tile_pool(name="g0p", bufs=2) as g0pool,
            tc.tile_pool(name="wbp", bufs=2) as wbpool,
            tc.tile_pool(name="w1p", bufs=2) as w1pool,
            tc.tile_pool(name="mkp", bufs=2) as mkpool,
            tc.tile_pool(name="pqr", bufs=3) as pqrpool,
            tc.tile_pool(name="ogp", bufs=2) as ogpool,
            tc.tile_pool(name="pmain", bufs=4, space="PSUM") as pmain,
        ):
            su32 = cpool.tile([128, 128], F32)
            sd32 = cpool.tile([128, 128], F32)
            su16 = cpool.tile([128, 128], F16)
            sd16 = cpool.tile([128, 128], F16)
            i16 = cpool.tile([128, 128], F16)
            i32 = cpool.tile([128, 128], F32)
            sum32 = cpool.tile([M_NP, M_NP], F32)
            sdm32 = cpool.tile([M_NP, M_NP], F32)
            sum16 = cpool.tile([M_NP, M_NP], F16)
            sdm16 = cpool.tile([M_NP, M_NP], F16)
            def load_consts():
                # Act-queue HWDGE: runs in parallel with the set-0 world
                # loads on SP instead of queueing behind them
                for t, d in ((su32, su32_d), (sd32, sd32_d), (su16, su16_d),
                             (sd16, sd16_d), (i16, i16_d), (i32, i32_d),
                             (sum32, sum32_d),
                             (sdm32, sdm32_d), (sum16, sum16_d),
                             (sdm16, sdm16_d)):
                    nc.scalar.dma_start(out=t[:], in_=d)
            ones16 = cpool.tile([128, W], F16)
            nc.gpsimd.memset(ones16[:], 1.0)

            def masks_rolls(st, step):
                """Phase 1: dA2 = roll(d2,+1,H) via fp32 matmul (step 2)."""
                sd = st.sd
                np_ = sd["np"]
                if step == 1:
                    d = st.d32[0:np_, :]
                    dA = st.dA1[0:np_, :]
                    return {"d": d, "dA": dA}
                d = st.w1d[0:np_, :]
                psx = pmain.tile([np_, 2 * W], F32, tag="ps")
                nc.tensor.matmul(out=psx[:, 0:W], lhsT=sd["su32"], rhs=d,
                                 start=True, stop=True)
                dAt = mkpool.tile([np_, W], F32, tag="dA2")
                nc.scalar.copy(dAt[:], psx[:, 0:W])
                return {"d": d, "dA": dAt[:], "psx": psx}

            def masks_cmps(st, step, mc):
                """Phase 2: density comparisons via shifted free-axis APs."""
                np_ = st.sd["np"]
                shift_w = 1 if step == 1 else -1
                d, dA = mc["d"], mc["dA"]
                c1 = mkpool.tile([np_, W], F16, tag="c1")
                c2 = mkpool.tile([np_, W], F16, tag="c2")
                c3 = mkpool.tile([np_, W], F16, tag="c3")
                _cmp_rolled(nc, al, c1, d, d, shift_w, al.is_ge)
                nc.vector.tensor_tensor(out=c2[:], in0=dA, in1=d, op=al.is_lt)
                _cmp_rolled(nc, al, c3, dA, d, shift_w, al.is_lt)
                mc.update(c1=c1, c2=c2, c3=c3)

            def masks_chain(st, step, mc):
                """Phase 3: AND-tree -> a16; b16 = roll(a,-1,H) via matmul."""
                sd = st.sd
                np_ = sd["np"]
                e = (st.wb if step == 1 else st.w1b)[0:np_, 0:W]
                e3c3 = mkpool.tile([np_, W], F16, tag="e3")
                nc.vector.scalar_tensor_tensor(out=e3c3[:], in0=e,
                                               scalar=ELEM_ID,
                                               in1=mc["c3"][:],
                                               op0=al.is_equal,
                                               op1=al.logical_and)
                c12 = mkpool.tile([np_, W], F16, tag="c12")
                nc.vector.tensor_tensor(out=c12[:], in0=mc["c1"][:],
                                        in1=mc["c2"][:], op=al.logical_and)
                mp = mkpool.tile([np_, 2 * W], F16, tag="mp")
                a16 = mp[:, 0:W]
                nc.vector.tensor_tensor(out=a16, in0=c12[:], in1=e3c3[:],
                                        op=al.logical_and)
                # b16[m] = a16[m+1]; sd16 zeroes the boundary rows natively
                psx = mc.get("psx")
                if psx is None:
                    psx = pmain.tile([np_, 2 * W], F32, tag="ps")
                    mc["psx"] = psx
                nc.tensor.matmul(out=psx[:, W : 2 * W], lhsT=sd["sd16"],
                                 rhs=a16, start=True, stop=True)
                b16 = mkpool.tile([np_, W], F16, tag="b16")
                nc.scalar.copy(b16[:], psx[:, W : 2 * W])
                mc.update(a16=a16, b16=b16, mp=mp)

            def masks_fin(st, step, mc):
                """Phase 4: r16/m016 (after the b16 DMA has had time)."""
                np_ = st.sd["np"]
                a16, b16 = mc["a16"], mc["b16"]
                r16 = mkpool.tile([np_, W], F16, tag="r16")
                nc.vector.tensor_tensor(out=r16[:], in0=a16, in1=b16[:],
                                        op=al.logical_or)
                m016 = mc["mp"][:, W : 2 * W]
                # r < 1 == (r == 0) for 0/1 masks; tensor_scalar -> 4x DVE
                nc.vector.tensor_scalar(out=m016, in0=r16[:], scalar1=1.0,
                                        scalar2=None, op0=al.is_lt)
                if step == 1:
                    st.m1 = (mc["mp"], b16)
                else:
                    st.m2 = (mc["mp"], b16)

            def exact1(st, mc):
                """Exact fp32 density path (feeds step-2 comparisons)."""
                sd = st.sd
                np_ = sd["np"]
                d = st.d32[0:np_, :]
                mp, b16 = st.m1
                a16 = mp[:, 0:W]
                m016 = mp[:, W : 2 * W]
                P0 = mkpool.tile([np_, W], F32, tag="P0")
                Q0 = mkpool.tile([np_, W], F32, tag="Q0")
                R0 = mkpool.tile([np_, W], F32, tag="R0")
                nc.vector.tensor_tensor(out=P0[:], in0=a16, in1=d,
                                        op=al.mult)
                nc.vector.tensor_tensor(out=Q0[:], in0=b16[:], in1=d,
                                        op=al.mult)
                nc.vector.tensor_tensor(out=R0[:], in0=m016, in1=d,
                                        op=al.mult)
                psd = pmain.tile([np_, 2 * W], F32, tag="ps")
                nc.tensor.matmul(out=psd[:, 0:W], lhsT=sd["su32"],
                                 rhs=Q0[:], start=True, stop=False)
                nc.tensor.matmul(out=psd[:, 0:W], lhsT=sd["sd32"],
                                 rhs=P0[:], start=False, stop=True)
                w1d = w1pool.tile([np_, W], F32, tag="w1d")
                nc.vector.tensor_tensor(out=w1d[:], in0=psd[:, 0:W],
                                        in1=R0[:], op=al.add)
                st.w1d = w1d

            def emit_group(st, step, g):
                """One fp16 stencil group-step: products, matmuls, evac."""
                sd = st.sd
                np_ = sd["np"]
                mp, b16 = st.m1 if step == 1 else st.m2
                src = (st.wb if step == 1 else st.w1b)[
                    0:np_, g * NCH * W : (g + 1) * NCH * W]
                src_v = src.rearrange("p (c w) -> p c w", c=NCH)
                b_b = b16[:].unsqueeze(1).broadcast_to([np_, NCH, W])
                fd = NCH * W
                mode = QMODE[(step, g)]
                if sd.get("last") and step == 2 and g >= 3:
                    mode = "dve"    # shorten the drain tail
                # Q first: the su-chain consumes it before P/R are needed
                Q = pqrpool.tile([np_, fd], F16, tag="Q")
                Qv = Q[:].rearrange("p (c w) -> p c w", c=NCH)
                h = NCH // 2
                if mode in ("split", "rpool"):
                    nc.gpsimd.tensor_tensor(
                        out=Qv[:, 0:h], in0=b_b[:, 0:h], in1=src_v[:, 0:h],
                        op=al.mult)
                    nc.vector.tensor_tensor(
                        out=Qv[:, h:NCH], in0=b_b[:, h:NCH],
                        in1=src_v[:, h:NCH], op=al.mult)
                    halves = ((2, 3), (0, 1))   # DVE-made half first
                elif mode == "pool":
                    # two half-ops so the first half's matmuls start sooner
                    nc.gpsimd.tensor_tensor(
                        out=Qv[:, 0:h], in0=b_b[:, 0:h], in1=src_v[:, 0:h],
                        op=al.mult)
                    nc.gpsimd.tensor_tensor(
                        out=Qv[:, h:NCH], in0=b_b[:, h:NCH],
                        in1=src_v[:, h:NCH], op=al.mult)
                    halves = ((0, 1), (2, 3))
                else:
                    nc.vector.tensor_tensor(out=Qv, in0=b_b, in1=src_v,
                                            op=al.mult)
                    halves = ((0, 1), (2, 3))
                PR = pqrpool.tile([np_, 2 * fd], F16, tag="PR")
                if mode == "rpool":
                    # P on DVE; R fully on Pool (I-chain consumes R last,
                    # so Pool's latency is tolerable)
                    a_b = mp[:, 0:W].unsqueeze(1).broadcast_to(
                        [np_, NCH, W])
                    m_b = mp[:, W : 2 * W].unsqueeze(1).broadcast_to(
                        [np_, NCH, W])
                    nc.vector.tensor_tensor(
                        out=PR[:, 0:fd].rearrange("p (c w) -> p c w", c=NCH),
                        in0=a_b, in1=src_v, op=al.mult)
                    nc.gpsimd.tensor_tensor(
                        out=PR[:, fd : 2 * fd].rearrange(
                            "p (c w) -> p c w", c=NCH),
                        in0=m_b, in1=src_v, op=al.mult)
                else:
                    # P and R as ONE double-wide DVE op: out [np,2,NCH,W],
                    # masks [a16 | m016] broadcast over channels, src
                    # broadcast over the P/R axis.
                    PRv = PR[:].rearrange("p (k c w) -> p k c w", k=2, c=NCH)
                    mp_b = mp.rearrange("p (k w) -> p k w", k=2).unsqueeze(
                        2).broadcast_to([np_, 2, NCH, W])
                    src_b = src_v.unsqueeze(1).broadcast_to([np_, 2, NCH, W])
                    nc.vector.tensor_tensor(out=PRv, in0=mp_b, in1=src_b,
                                            op=al.mult)
                P = PR[:, 0:fd]
                R = PR[:, fd : 2 * fd]
                if step == 1:
                    og = None
                    dst = st.w1b[0:np_, g * fd : (g + 1) * fd]
                else:
                    og = ogpool.tile([np_, fd], F16, tag="og")
                    dst = og[0:np_, :]
                hw = 2 * W
                for (c0, c1) in halves:
                    ps = pmain.tile([np_, hw], F32, tag="ps")
                    for c in (c0, c1):
                        r = slice((c - c0) * W, (c - c0 + 1) * W)
                        nc.tensor.matmul(out=ps[:, r], lhsT=sd["su16"],
                                         rhs=Q[:, c * W : (c + 1) * W],
                                         start=True, stop=False)
                        nc.tensor.matmul(out=ps[:, r], lhsT=sd["sd16"],
                                         rhs=P[:, c * W : (c + 1) * W],
                                         start=False, stop=False)
                        nc.tensor.matmul(out=ps[:, r], lhsT=sd["i16"],
                                         rhs=R[:, c * W : (c + 1) * W],
                                         start=False, stop=True)
                    nc.scalar.copy(dst[:, c0 * W : (c1 + 1) * W], ps[:])
                if og is not None:
                    sd["store"](og, g)

            def prep_A(sd):
                """Next-set loads: density tiles first (they gate the
                mask pipeline), then the fp16 world in one DMA."""
                st = SetCtx(sd)
                st.d32 = g0pool.tile([128, W], F32, tag="d32")
                sd["load_d"](st.d32)
                st.dA1 = g0pool.tile([128, W], F32, tag="dA1")
                sd["load_dA1"](st.dA1)
                st.wb = wbpool.tile([128, C * W], F16, tag="wb")
                sd["load_w"](st.wb)
                return st

            def prep_B(st, mix_dve=False):
                """fp16 world arrives converted from the host: nothing to
                stage or convert."""

            def make_main_set(bi, si):
                r_out = si * MAIN_OUT

                def load_w(t):
                    _load_rows(nc, t, wd16[bi], r_out - 2, 128, nch=C)

                def load_d(t):
                    _load_rows_2d(nc, t, wd32[bi], r_out - 2, 128)

                def load_dA1(t):
                    _load_rows_2d(nc, t, wd32[bi], r_out - 3, 128)

                def store(og, g):
                    dst = od[bi, g * NCH : (g + 1) * NCH,
                             r_out : r_out + MAIN_OUT, :]
                    nc.scalar.dma_start(
                        out=dst.rearrange("c h w -> h c w"),
                        in_=og[2 : 2 + MAIN_OUT, :].rearrange(
                            "h (c w) -> h c w", c=NCH))

                return {"np": 128, "su32": su32[:], "sd32": sd32[:],
                        "su16": su16[:], "sd16": sd16[:],
                        "i16": i16[:], "i32": i32[:], "blocks": [(0, 128)],
                        "load_w": load_w, "load_d": load_d,
                        "load_dA1": load_dA1, "store": store}

            def make_merged_set():
                r_out = 4 * MAIN_OUT
                n_out = H - r_out        # 16
                blocks = [(0, n_out + 4), (M_B1, n_out + 4)]

                def load_w(t):
                    nc.gpsimd.memset(t[0:64, :], 0.0)
                    for bi, p0 in ((0, 0), (1, M_B1)):
                        _load_rows(nc, t, wd16[bi], r_out - 2, n_out + 4,
                                   p0=p0, nch=C)

                def load_d(t):
                    nc.gpsimd.memset(t[0:64, :], 0.0)
                    for bi, p0 in ((0, 0), (1, M_B1)):
                        _load_rows_2d(nc, t, wd32[bi], r_out - 2, n_out + 4,
                                      p0=p0)

                def load_dA1(t):
                    nc.gpsimd.memset(t[0:64, :], 0.0)
                    for bi, p0 in ((0, 0), (1, M_B1)):
                        _load_rows_2d(nc, t, wd32[bi], r_out - 3, n_out + 4,
                                      p0=p0)

                def store(og, g):
                    for bi, p0 in ((0, 2), (1, M_B1 + 2)):
                        dst = od[bi, g * NCH : (g + 1) * NCH,
                                 r_out : r_out + n_out, :]
                        nc.scalar.dma_start(
                            out=dst.rearrange("c h w -> h c w"),
                            in_=og[p0 : p0 + n_out, :].rearrange(
                                "h (c w) -> h c w", c=NCH))

                return {"np": M_NP, "su32": sum32[:], "sd32": sdm32[:],
                        "su16": sum16[:], "sd16": sdm16[:],
                        "i16": i16[0:M_NP, 0:M_NP],
                        "i32": i32[0:M_NP, 0:M_NP], "blocks": blocks,
                        "last": True,
                        "load_w": load_w, "load_d": load_d,
                        "load_dA1": load_dA1, "store": store}

            sets = [make_main_set(bi, si)
                    for bi in range(B_PER_CORE) for si in range(4)]
            sets.append(make_merged_set())

            # deep software pipeline. Per-engine queues run in emission
            # order, so long-latency chains (partition-shift DMAs, evac-
            # dependent compares) are split into phases and interleaved
            # between bulk product groups that hide their latency.
            st = prep_A(sets[0])
            prep_B(st, mix_dve=True)
            load_consts()
            mc1 = masks_rolls(st, 1)
            masks_cmps(st, 1, mc1)
            masks_chain(st, 1, mc1)
            masks_fin(st, 1, mc1)
            exact1(st, mc1)
            st.w1b = w1pool.tile([128, C * W], F16, tag="w1b")
            for i in range(len(sets)):
                # phase B: step 1 with step-2 mask phases interleaved
                emit_group(st, 1, 0)
                mc2 = masks_rolls(st, 2)
                masks_cmps(st, 2, mc2)
                emit_group(st, 1, 1)
                masks_chain(st, 2, mc2)
                emit_group(st, 1, 2)
                masks_fin(st, 2, mc2)
                emit_group(st, 1, 3)
                emit_group(st, 1, 4)
                # phase C: step 2 with next-set prep interleaved
                nxt = sets[i + 1] if i + 1 < len(sets) else None
                stn = prep_A(nxt) if nxt else None
                emit_group(st, 2, 0)
                if stn:
                    prep_B(stn)
                emit_group(st, 2, 1)
                if stn:
                    mc1 = masks_rolls(stn, 1)
                    masks_cmps(stn, 1, mc1)
                emit_group(st, 2, 2)
                if stn:
                    masks_chain(stn, 1, mc1)
                    masks_fin(stn, 1, mc1)
                emit_group(st, 2, 3)
                if stn:
                    exact1(stn, mc1)
                    stn.w1b = w1pool.tile([128, C * W], F16, tag="w1b")
                emit_group(st, 2, 4)
                st = stn

    nc.compile()
    return nc


def _shift_mats():
    su = np.zeros((128, 128), np.float32)   # out[m] = in[m-1]
    sdn = np.zeros((128, 128), np.float32)  # out[m] = in[m+1]
    for m in range(128):
        if m >= 1:
            su[m - 1, m] = 1.0
        if m <= 126:
            sdn[m + 1, m] = 1.0
    sum_ = np.zeros((M_NP, M_NP), np.float32)
    sdm = np.zeros((M_NP, M_NP), np.float32)
    for base in (0, M_B1):
        for m in range(20):
            if m >= 1:
                sum_[base + m - 1, base + m] = 1.0
            if m <= 18:
                sdm[base + m + 1, base + m] = 1.0
    return su, sdn, sum_, sdm


_NC_CACHE = {}


def kernel(world, rand_movement=None, rand_interact=None, rand_element=None,
           **_ignored):
    world = np.ascontiguousarray(world, dtype=np.float32)
    assert world.shape == (B, C, H, W), world.shape
    if "nc" not in _NC_CACHE:
        _NC_CACHE["nc"] = build_kernel()
    nc = _NC_CACHE["nc"]
    su, sdn, sum_, sdm = _shift_mats()
    i16 = np.eye(128, dtype=np.float16)
    world16 = world.astype(np.float16)
    in_maps = []
    for core in range(N_CORES):
        sl = slice(core * B_PER_CORE, (core + 1) * B_PER_CORE)
        in_maps.append({
            "world16": np.ascontiguousarray(world16[sl]),
            "wd32": np.ascontiguousarray(world[sl][:, 1]),
            "su32": su, "sd32": sdn,
            "su16": su.astype(np.float16), "sd16": sdn.astype(np.float16),
            "i16": i16, "i32": np.eye(128, dtype=np.float32),
            "sum32": sum_, "sdm32": sdm,
            "sum16": sum_.astype(np.float16),
            "sdm16": sdm.astype(np.float16),
        })
    res = run_bass_kernel_spmd(nc, in_maps, list(range(N_CORES)),
                               trace=_NC_CACHE.get("trace", False))
    _NC_CACHE["last_result"] = res
    out = np.concatenate([r["out16"] for r in res.results], axis=0)
    return out.astype(np.float32)


if __name__ == "__main__":
    rng = np.random.default_rng(0)
    w = rng.standard_normal((B, C, H, W)).astype(np.float32)
    w[:, 0] = rng.integers(0, 10, (B, H, W)).astype(np.float32)
    out = kernel(w)
    print("ran:", out.shape, out.dtype)
